# revision 21
# baseline (speedup 1.0000x reference)
"""Performer attention TRN2 Bass kernel.

Strategy: sequence-parallel over the 8 cores (each core owns 2048 rows =
half of one batch; cores 2i,2i+1 share batch i). The Performer kv
aggregation sums over the full sequence, so the two cores of a pair
AllGather their partial kv matrices (tiny: H*65*R fp32 ~ 1MB) and sum.
Everything else is fully local.

Math restructuring vs the reference (exactly equivalent in real
arithmetic): q_prime is computed WITHOUT the -0.5|q|^2 stabilizer; the
factor e^{-qsq} cancels between numerator and normalizer, except in the
+1e-6 term, which is compensated by using denominator
(Nu + 1e-6 * e^{qsq}).  k_prime keeps its stabilizer (it is inside the
sequence sum).

All matmuls run as float32r (fp22 multiplies, fp32 accumulate) which is
full PE speed for moving dim >= 256.

Layouts (host pre-transposed, see kernel()):
  xT   [D, MLOC]   feature-major activations
  w*T  [D, D]      transposed weights
  rfa2 [D, 2*(R+HD)] per d-tile j: block-diag [rf_{2j}|I64] / [rf_{2j+1}|I64]
  rfq2 [D, 2*R]      per d-tile j: block-diag rf_{2j} / rf_{2j+1}
"""

import os
import sys

import numpy as np

for _p in ("/opt/trn_rl_repo", "/opt/pypackages"):
    if _p not in sys.path:
        sys.path.append(_p)

B, S, D, H, R, HD = 4, 4096, 1024, 16, 256, 64
NCORES = 8
MLOC = (B * S) // NCORES  # 2048

FULL_CFG = dict(
    D=D, H=H, R=R, HD=HD, MLOC=MLOC, CHUNK=512,
    ncores=NCORES, pairs=[[0, 1], [2, 3], [4, 5], [6, 7]],
)


def _emit(tc, io, c):
    import concourse.bass as bass  # noqa: F401
    from concourse import mybir

    nc = tc.nc
    f32 = mybir.dt.float32
    f32r = mybir.dt.float32r
    AF = mybir.ActivationFunctionType
    OP = mybir.AluOpType
    P = 128

    D_, H_, R_, HD_ = c["D"], c["H"], c["R"], c["HD"]
    M_, CH = c["MLOC"], c["CHUNK"]
    ND = D_ // P          # d-tiles (= head pairs)
    NM = CH // P          # m-tiles per chunk
    NC_ = M_ // CH        # chunks
    RT = R_ // P          # r-tiles per head
    NW = c.get("NW", min(512, D_))  # n-chunk width
    NNCH = D_ // NW       # n chunks
    W65 = HD_ + 1         # 65
    SQH = float(np.sqrt(0.5))
    LNEPS = float(np.log(1e-6))

    def r(ap):
        return ap.bitcast(f32r)

    mm = nc.tensor.matmul

    xT, wqT, wkT, wvT, woT = io["xT"], io["wqT"], io["wkT"], io["wvT"], io["woT"]
    rfa2, rfq2, hm_ab = io["rfa2"], io["rfq2"], io["hm_ab"]
    bq_t, bk_t, bv_bc, bo_bc = io["bq_t"], io["bk_t"], io["bv_bc"], io["bo_bc"]
    out_d = io["out"]

    from contextlib import ExitStack
    with (
        tc.tile_pool(name="const", bufs=1) as p_const,
        tc.tile_pool(name="kvrp", bufs=1) as p_kvr,
        tc.tile_pool(name="wq", bufs=ND) as p_wq,
        tc.tile_pool(name="psum", bufs=8, space="PSUM") as pp,
        tc.tile_pool(name="dram", bufs=1, space="DRAM") as p_dram,
    ):
        # ---- persistent constants ----
        id_sb = p_const.tile([P, P], f32, tag="ident")
        from concourse.masks import make_identity
        make_identity(nc, id_sb[:])
        hm_sb = p_const.tile([P, 2], f32r, tag="hm")
        nc.sync.dma_start(hm_sb[:], hm_ab[:, :])
        bq_sb = p_const.tile([P, ND], f32, tag="bq")
        nc.sync.dma_start(bq_sb[:], bq_t[:, :])
        bk_sb = p_const.tile([P, ND], f32, tag="bk")
        nc.sync.dma_start(bk_sb[:], bk_t[:, :])
        bv_sb = p_const.tile([P, H_ * W65], f32, tag="bv")
        nc.sync.dma_start(bv_sb[:], bv_bc[:, :])
        bo_sb = p_const.tile([P, D_], f32, tag="bo")
        nc.sync.dma_start(bo_sb[:], bo_bc[:, :])
        lneps_sb = p_const.tile([1, 1], f32, tag="lneps")
        nc.gpsimd.memset(lneps_sb[:], LNEPS)
        wq_sb = []
        for t in range(ND):
            w1 = p_wq.tile([P, D_], f32r, tag="wq")
            nc.sync.dma_start(w1[:], wqT[t * P:(t + 1) * P, :])
            wq_sb.append(w1)

        # kv accumulator [65, H*R] — scoped so it frees after the DMA out
        cc_in = p_dram.tile([W65, H_ * R_], f32, tag="ccin")
        cc_out = p_dram.tile([2 * W65, H_ * R_], f32, tag="ccout")
        with (
            tc.tile_pool(name="kvloc", bufs=1) as p_kvloc,
            tc.tile_pool(name="wk", bufs=ND) as p_wk,
            tc.tile_pool(name="wv", bufs=ND) as p_wv,
            tc.tile_pool(name="rfa", bufs=ND) as p_rfa,
            tc.tile_pool(name="xk", bufs=ND) as p_x,
            tc.tile_pool(name="ktc", bufs=ND) as p_kt,
            tc.tile_pool(name="vt", bufs=NM) as p_v,
            tc.tile_pool(name="kp", bufs=4) as p_kp,
            tc.tile_pool(name="ksm", bufs=6) as p_ksm,
        ):
            kv_sb = p_kvloc.tile([W65, H_ * R_], f32, tag="kvloc")
            nc.gpsimd.memset(kv_sb[:], 0.0)
            wk_sb = []
            wv_sb = []
            rfa_sb = []
            for t in range(ND):
                w1 = p_wk.tile([P, D_], f32r, tag="wk")
                nc.sync.dma_start(w1[:], wkT[t * P:(t + 1) * P, :])
                wk_sb.append(w1)
                w2 = p_wv.tile([P, D_], f32r, tag="wv")
                nc.sync.dma_start(w2[:], wvT[t * P:(t + 1) * P, :])
                wv_sb.append(w2)
                rr = p_rfa.tile([P, 2 * (R_ + HD_)], f32r, tag="rfa")
                nc.sync.dma_start(rr[:], rfa2[t * P:(t + 1) * P, :])
                rfa_sb.append(rr)

            for ch in range(NC_):
                c0 = ch * CH
                x_sb = []
                for t in range(ND):
                    xt = p_x.tile([P, CH], f32r, tag="xk")
                    nc.sync.dma_start(xt[:], xT[t * P:(t + 1) * P, c0:c0 + CH])
                    x_sb.append(xt)
                # kT projection (feature-major)
                kt_sb = []
                for nt in range(ND):
                    ps = pp.tile([P, CH], f32, tag="ps")
                    for kt in range(ND):
                        mm(ps[:], r(wk_sb[kt][:, nt * P:(nt + 1) * P]),
                           r(x_sb[kt][:]), start=(kt == 0), stop=(kt == ND - 1))
                    ktt = p_kt.tile([P, CH], f32r, tag="ktc")
                    nc.vector.tensor_scalar_add(ktt[:], ps[:], bk_sb[:, nt:nt + 1])
                    kt_sb.append(ktt)
                # v projection (seq-major, scattered into 65-wide head slots)
                v_sb = []
                for mt in range(NM):
                    vt = p_v.tile([P, H_ * W65], f32r, tag="vt")
                    for nch in range(NNCH):
                        ps = pp.tile([P, NW], f32, tag="ps")
                        for kt in range(ND):
                            mm(ps[:], r(x_sb[kt][:, mt * P:(mt + 1) * P]),
                               r(wv_sb[kt][:, nch * NW:(nch + 1) * NW]),
                               start=(kt == 0), stop=(kt == ND - 1))
                        hpc = NW // HD_  # heads per n-chunk (8)
                        ov = vt[:].rearrange("p (h w) -> p h w", w=W65)[
                            :, nch * hpc:(nch + 1) * hpc, 0:HD_]
                        iv = ps[:].rearrange("p (h w) -> p h w", w=HD_)
                        bb = bv_sb[:].rearrange("p (h w) -> p h w", w=W65)[
                            :, nch * hpc:(nch + 1) * hpc, 0:HD_]
                        nc.vector.tensor_tensor(ov, iv, bb, OP.add)
                    # ones columns (from bv_bc, which holds 1.0 at slot col 64)
                    oo = vt[:].rearrange("p (h w) -> p h w", w=W65)[:, :, HD_:W65]
                    bo1 = bv_sb[:].rearrange("p (h w) -> p h w", w=W65)[:, :, HD_:W65]
                    nc.vector.tensor_copy(oo, bo1)
                    v_sb.append(vt)
                # heads: features, exp, kv accumulation
                for j in range(ND):
                    for hh in range(2):
                        h = 2 * j + hh
                        kvp = pp.tile([W65, R_], f32, tag="ps")
                        for mt in range(NM):
                            kfa = pp.tile([P, R_ + HD_], f32, tag="ps")
                            mm(kfa[:], r(kt_sb[j][:, mt * P:(mt + 1) * P]),
                               r(rfa_sb[j][:, hh * (R_ + HD_):(hh + 1) * (R_ + HD_)]),
                               start=True, stop=True)
                            sqs = p_ksm.tile([P, HD_], f32, tag="sqs")
                            ksq = p_ksm.tile([P, 1], f32, tag="ksq")
                            nc.scalar.activation(sqs[:], kfa[:, R_:R_ + HD_],
                                                 AF.Square, scale=SQH,
                                                 accum_out=ksq[:])
                            nksq = p_ksm.tile([P, 1], f32, tag="nksq")
                            nc.gpsimd.tensor_scalar_mul(nksq[:], ksq[:], -1.0)
                            kpt = p_kp.tile([P, R_], f32r, tag="kp")
                            nc.scalar.activation(kpt[:], kfa[:, 0:R_], AF.Exp,
                                                 bias=nksq[:, 0:1])
                            mm(kvp[:], r(v_sb[mt][:, h * W65:(h + 1) * W65]),
                               r(kpt[:]), start=(mt == 0), stop=(mt == NM - 1))
                        nc.vector.tensor_tensor(
                            kv_sb[:, h * R_:(h + 1) * R_],
                            kv_sb[:, h * R_:(h + 1) * R_], kvp[:], OP.add)

            nc.sync.dma_start(cc_in[:], kv_sb[:])

        # =================== PAIR ALLGATHER + TRANSPOSE ===================
        nc.gpsimd.collective_compute(
            "AllGather", OP.bypass, replica_groups=c["pairs"],
            ins=[cc_in[:].opt()], outs=[cc_out[:].opt()])
        kvr = p_kvr.tile([P, H_ * RT * W65], f32r, tag="kvr")
        with tc.tile_pool(name="kvg", bufs=1) as p_kvg:
            kva = p_kvg.tile([W65, H_ * R_], f32, tag="kva")
            nc.sync.dma_start(kva[:], cc_out[0:W65, :])
            kvb = p_kvg.tile([W65, H_ * R_], f32, tag="kvb")
            nc.sync.dma_start(kvb[:], cc_out[W65:2 * W65, :])
            nc.vector.tensor_tensor(kva[:], kva[:], kvb[:], OP.add)
            # transpose to r-major: kvr [128(r), (2h+rt)*65]
            for h in range(H_):
                tp = pp.tile([P, RT * W65], f32, tag="ps")
                for rt in range(RT):
                    nc.tensor.transpose(
                        tp[:, rt * W65:(rt + 1) * W65],
                        kva[:, h * R_ + rt * P: h * R_ + (rt + 1) * P],
                        id_sb[0:W65, 0:W65])
                nc.vector.tensor_copy(
                    kvr[:, (RT * h) * W65:(RT * h + RT) * W65], tp[:])

        # =================== PASS Q ===================
        if True:
            with (
                tc.tile_pool(name="rfq", bufs=ND) as p_rfq,
                tc.tile_pool(name="wo", bufs=ND) as p_wo,
                tc.tile_pool(name="xq", bufs=ND + 2) as p_xq,
                tc.tile_pool(name="qtc", bufs=ND) as p_qt,
                tc.tile_pool(name="q2", bufs=3) as p_q2,
                tc.tile_pool(name="qp", bufs=5) as p_qp,
                tc.tile_pool(name="att", bufs=ND) as p_att,
                tc.tile_pool(name="nrm", bufs=3) as p_nrm,
                tc.tile_pool(name="bcp", bufs=2) as p_bc,
                tc.tile_pool(name="oub", bufs=2) as p_ou,
            ):
                rfq_sb = []
                wo_sb = []
                for t in range(ND):
                    rr = p_rfq.tile([P, 2 * R_], f32r, tag="rfq")
                    nc.sync.dma_start(rr[:], rfq2[t * P:(t + 1) * P, :])
                    rfq_sb.append(rr)
                for t in range(ND):
                    w2 = p_wo.tile([P, D_], f32r, tag="wo")
                    nc.sync.dma_start(w2[:], woT[t * P:(t + 1) * P, :])
                    wo_sb.append(w2)

                for ch in range(NC_):
                    c0 = ch * CH
                    x_sb = []
                    for t in range(ND):
                        xt = p_xq.tile([P, CH], f32r, tag="xq")
                        nc.sync.dma_start(xt[:], xT[t * P:(t + 1) * P, c0:c0 + CH])
                        x_sb.append(xt)
                    qt_sb = []
                    for nt in range(ND):
                        ps = pp.tile([P, CH], f32, tag="ps")
                        for kt in range(ND):
                            mm(ps[:], r(wq_sb[kt][:, nt * P:(nt + 1) * P]),
                               r(x_sb[kt][:]), start=(kt == 0), stop=(kt == ND - 1))
                        qtt = p_qt.tile([P, CH], f32r, tag="qtc")
                        nc.scalar.add(qtt[:], ps[:], bq_sb[:, nt:nt + 1])
                        qt_sb.append(qtt)
                    attn_sb = []
                    for j in range(ND):
                        q2t = p_q2.tile([P, CH], f32r, tag="q2")
                        nc.scalar.activation(q2t[:], qt_sb[j][:], AF.Square)
                        att = p_att.tile([P, CH], f32r, tag="att")
                        for hh in range(2):
                            h = 2 * j + hh
                            # q features + exp (no stabilizer)
                            qp_t = []
                            for rt in range(RT):
                                qf = pp.tile([P, CH], f32, tag="ps")
                                mm(qf[:], r(rfq_sb[j][:, hh * R_ + rt * P:
                                                      hh * R_ + (rt + 1) * P]),
                                   r(qt_sb[j][:]), start=True, stop=True)
                                qp = p_qp.tile([P, CH], f32r, tag="qp")
                                nc.scalar.activation(qp[:], qf[:], AF.Exp)
                                qp_t.append(qp)
                            # qsq -> eqsq = 1e-6 * e^{qsq}; park it at partition 64
                            qsq = pp.tile([1, CH], f32, tag="ps")
                            mm(qsq[:], r(hm_sb[:, hh:hh + 1]), r(q2t[:]),
                               start=True, stop=True)
                            eqsq = p_nrm.tile([1, CH], f32, tag="eqsq")
                            nc.scalar.activation(eqsq[:], qsq[:], AF.Exp,
                                                 bias=lneps_sb[0:1, 0:1])
                            # attention + normalizer in one matmul (M=65)
                            ah = pp.tile([W65, CH], f32, tag="ps")
                            for rt in range(RT):
                                mm(ah[:],
                                   r(kvr[:, (RT * h + rt) * W65:
                                         (RT * h + rt) * W65 + W65]),
                                   r(qp_t[rt][:]), start=(rt == 0),
                                   stop=(rt == RT - 1))
                            # denom+recip at partition 64, then DMA the row
                            # down to partition 0 for the broadcast
                            e64 = p_nrm.tile([W65, CH], f32, tag="e64")
                            nc.sync.dma_start(e64[HD_:W65, :], eqsq[:])
                            nc.vector.tensor_tensor(e64[HD_:W65, :],
                                                    ah[HD_:W65, :],
                                                    e64[HD_:W65, :], OP.add)
                            nc.vector.reciprocal(e64[HD_:W65, :], e64[HD_:W65, :])
                            r0 = p_nrm.tile([1, CH], f32, tag="r0")
                            nc.sync.dma_start(r0[:], e64[HD_:W65, :])
                            bcp = p_bc.tile([HD_, CH], f32, tag="bcp")
                            nc.gpsimd.partition_broadcast(bcp[:], r0[:])
                            mul32 = p_bc.tile([HD_, CH], f32, tag="mul32")
                            nc.vector.tensor_tensor(mul32[:], ah[0:HD_, :],
                                                    bcp[:], OP.mult)
                            if hh == 0:
                                nc.scalar.copy(att[0:HD_, :], mul32[:])
                            else:
                                scr = p_bc.tile([HD_, CH], f32r, tag="scr")
                                nc.scalar.copy(scr[:], mul32[:])
                                # partition move 0:64 -> 64:128 via DMA
                                nc.sync.dma_start(att[HD_:P, :], scr[:])
                        attn_sb.append(att)
                    # out projection (seq-major) + bias + store
                    for mt in range(NM):
                        ot = p_ou.tile([P, D_], f32, tag="oub")
                        for nch in range(NNCH):
                            ps = pp.tile([P, NW], f32, tag="ps")
                            for j in range(ND):
                                mm(ps[:], r(attn_sb[j][:, mt * P:(mt + 1) * P]),
                                   r(wo_sb[j][:, nch * NW:(nch + 1) * NW]),
                                   start=(j == 0), stop=(j == ND - 1))
                            nc.vector.tensor_tensor(
                                ot[:, nch * NW:(nch + 1) * NW], ps[:],
                                bo_sb[:, nch * NW:(nch + 1) * NW], OP.add)
                        nc.sync.dma_start(
                            out_d[c0 + mt * P:c0 + (mt + 1) * P, :], ot[:])


def _declare_io(nc, c):
    from concourse import mybir
    f32 = mybir.dt.float32
    D_, H_, R_, HD_, M_ = c["D"], c["H"], c["R"], c["HD"], c["MLOC"]
    f32r = mybir.dt.float32r
    io = {}
    def inp(name, shape, dt=f32):
        io[name] = nc.dram_tensor(name, shape, dt, kind="ExternalInput").ap()
    inp("xT", [D_, M_], f32r)
    inp("wqT", [D_, D_], f32r)
    inp("wkT", [D_, D_], f32r)
    inp("wvT", [D_, D_], f32r)
    inp("woT", [D_, D_], f32r)
    inp("rfa2", [D_, 2 * (R_ + HD_)], f32r)
    inp("rfq2", [D_, 2 * R_], f32r)
    inp("hm_ab", [128, 2], f32r)
    inp("bq_t", [128, D_ // 128])
    inp("bk_t", [128, D_ // 128])
    inp("bv_bc", [128, H_ * (HD_ + 1)])
    inp("bo_bc", [128, D_])
    io["out"] = nc.dram_tensor("out", [M_, D_], f32, kind="ExternalOutput").ap()
    return io


def build_bass(cfg):
    import concourse.tile as tile
    from concourse import bacc
    nc = bacc.Bacc("TRN2", target_bir_lowering=False, debug=False,
                   enable_asserts=False, num_devices=cfg["ncores"])
    io = _declare_io(nc, cfg)
    with tile.TileContext(nc) as tc:
        _emit(tc, io, cfg)
    nc.compile()
    return nc


def host_inputs(x_slice_T, wq, bq, wk, bk, wv, bv, wo, bo, rf, cfg):
    """Build the per-core input map. x_slice_T: [D, MLOC] for this core."""
    D_, H_, R_, HD_ = cfg["D"], cfg["H"], cfg["R"], cfg["HD"]
    ND = D_ // 128
    f = np.float32
    ey = np.eye(HD_, dtype=f)
    rfa2 = np.zeros((D_, 2 * (R_ + HD_)), f)
    rfq2 = np.zeros((D_, 2 * R_), f)
    for j in range(ND):
        for hh in range(2):
            h = 2 * j + hh
            rows = slice(j * 128 + hh * HD_, j * 128 + (hh + 1) * HD_)
            rfa2[rows, hh * (R_ + HD_):hh * (R_ + HD_) + R_] = rf[h]
            rfa2[rows, hh * (R_ + HD_) + R_:(hh + 1) * (R_ + HD_)] = ey
            rfq2[rows, hh * R_:(hh + 1) * R_] = rf[h]
    hm_ab = np.zeros((128, 2), f)
    hm_ab[0:HD_, 0] = 0.5
    hm_ab[HD_:128, 1] = 0.5
    bv_bc = np.zeros((128, H_ * (HD_ + 1)), f)
    for h in range(H_):
        bv_bc[:, h * (HD_ + 1):h * (HD_ + 1) + HD_] = bv[h * HD_:(h + 1) * HD_][None, :]
        bv_bc[:, h * (HD_ + 1) + HD_] = 1.0
    return {
        "xT": np.ascontiguousarray(x_slice_T, f),
        "wqT": np.ascontiguousarray(wq.T, f),
        "wkT": np.ascontiguousarray(wk.T, f),
        "wvT": np.ascontiguousarray(wv.T, f),
        "woT": np.ascontiguousarray(wo.T, f),
        "rfa2": rfa2, "rfq2": rfq2, "hm_ab": hm_ab,
        "bq_t": np.ascontiguousarray(bq.reshape(ND, 128).T, f),
        "bk_t": np.ascontiguousarray(bk.reshape(ND, 128).T, f),
        "bv_bc": bv_bc,
        "bo_bc": np.ascontiguousarray(np.tile(bo[None, :], (128, 1)), f),
    }


_NC_CACHE = {}
LAST_RESULTS = None


def kernel(**inputs):
    global LAST_RESULTS
    from concourse.bass_utils import run_bass_kernel_spmd

    cfg = FULL_CFG
    x = np.asarray(inputs["x"], np.float32)
    args = [np.asarray(inputs[k], np.float32) for k in
            ["wq", "bq", "wk", "bk", "wv", "bv", "wo", "bo", "random_features"]]

    key = "full"
    if key not in _NC_CACHE:
        _NC_CACHE[key] = build_bass(cfg)
    nc = _NC_CACHE[key]

    base = host_inputs(np.zeros((cfg["D"], cfg["MLOC"]), np.float32), *args, cfg)
    in_maps = []
    for c in range(cfg["ncores"]):
        b, half = c // 2, c % 2
        m = dict(base)
        m["xT"] = np.ascontiguousarray(
            x[b, half * cfg["MLOC"]:(half + 1) * cfg["MLOC"], :].T)
        in_maps.append(m)

    trace = os.environ.get("KBENCH_TRACE", "0") == "1"
    res = run_bass_kernel_spmd(
        nc, in_maps, core_ids=list(range(cfg["ncores"])), trace=trace)
    LAST_RESULTS = res
    out = np.concatenate([res.results[c]["out"] for c in range(cfg["ncores"])],
                         axis=0)
    return out.reshape(B, S, D).astype(np.float32)


# revision 22
# speedup vs baseline: 1.1182x; 1.1182x over previous
"""Performer attention TRN2 Bass kernel.

Strategy: sequence-parallel over the 8 cores (each core owns 2048 rows =
half of one batch; cores 2i,2i+1 share batch i). The Performer kv
aggregation sums over the full sequence, so the two cores of a pair
AllGather their partial kv matrices (tiny: H*65*R fp32 ~ 1MB) and sum.
Everything else is fully local.

Math restructuring vs the reference (exactly equivalent in real
arithmetic): q_prime is computed WITHOUT the -0.5|q|^2 stabilizer; the
factor e^{-qsq} cancels between numerator and normalizer, except in the
+1e-6 term, which is compensated by using denominator
(Nu + 1e-6 * e^{qsq}).  k_prime keeps its stabilizer (it is inside the
sequence sum).

All matmuls run as float32r (fp22 multiplies, fp32 accumulate) which is
full PE speed for moving dim >= 256.

Layouts (host pre-transposed, see kernel()):
  xT   [D, MLOC]   feature-major activations
  w*T  [D, D]      transposed weights
  rfa2 [D, 2*(R+HD)] per d-tile j: block-diag [rf_{2j}|I64] / [rf_{2j+1}|I64]
  rfq2 [D, 2*R]      per d-tile j: block-diag rf_{2j} / rf_{2j+1}
"""

import os
import sys

import numpy as np

for _p in ("/opt/trn_rl_repo", "/opt/pypackages"):
    if _p not in sys.path:
        sys.path.append(_p)

B, S, D, H, R, HD = 4, 4096, 1024, 16, 256, 64
NCORES = 8
MLOC = (B * S) // NCORES  # 2048

FULL_CFG = dict(
    D=D, H=H, R=R, HD=HD, MLOC=MLOC, CHUNK=512,
    ncores=NCORES, pairs=[[0, 1], [2, 3], [4, 5], [6, 7]],
)


def _emit(tc, io, c):
    import concourse.bass as bass  # noqa: F401
    from concourse import mybir

    nc = tc.nc
    f32 = mybir.dt.float32
    f32r = mybir.dt.float32r
    AF = mybir.ActivationFunctionType
    OP = mybir.AluOpType
    P = 128

    D_, H_, R_, HD_ = c["D"], c["H"], c["R"], c["HD"]
    M_, CH = c["MLOC"], c["CHUNK"]
    ND = D_ // P          # d-tiles (= head pairs)
    NM = CH // P          # m-tiles per chunk
    NC_ = M_ // CH        # chunks
    RT = R_ // P          # r-tiles per head
    NW = c.get("NW", min(512, D_))  # n-chunk width
    NNCH = D_ // NW       # n chunks
    W65 = HD_ + 1         # 65
    SQH = float(np.sqrt(0.5))
    LNEPS = float(np.log(1e-6))

    def r(ap):
        return ap.bitcast(f32r)

    mm = nc.tensor.matmul

    xT, wqT, wkT, wvT, woT = io["xT"], io["wqT"], io["wkT"], io["wvT"], io["woT"]
    rfa2, rfq2, hm_ab = io["rfa2"], io["rfq2"], io["hm_ab"]
    bq_t, bk_t, bv_bc, bo_bc = io["bq_t"], io["bk_t"], io["bv_bc"], io["bo_bc"]
    out_d = io["out"]

    from contextlib import ExitStack
    with (
        tc.tile_pool(name="const", bufs=1) as p_const,
        tc.tile_pool(name="kvrp", bufs=1) as p_kvr,
        tc.tile_pool(name="wq", bufs=ND) as p_wq,
        tc.tile_pool(name="psum", bufs=8, space="PSUM") as pp,
        tc.tile_pool(name="dram", bufs=1, space="DRAM") as p_dram,
    ):
        # ---- persistent constants ----
        id_sb = p_const.tile([P, P], f32, tag="ident")
        from concourse.masks import make_identity
        make_identity(nc, id_sb[:])
        hm_sb = p_const.tile([P, 2], f32r, tag="hm")
        nc.sync.dma_start(hm_sb[:], hm_ab[:, :])
        bq_sb = p_const.tile([P, ND], f32, tag="bq")
        nc.sync.dma_start(bq_sb[:], bq_t[:, :])
        bk_sb = p_const.tile([P, ND], f32, tag="bk")
        nc.sync.dma_start(bk_sb[:], bk_t[:, :])
        bv_sb = p_const.tile([P, H_ * W65], f32, tag="bv")
        nc.sync.dma_start(bv_sb[:], bv_bc[:, :])
        bo_sb = p_const.tile([P, D_], f32, tag="bo")
        nc.sync.dma_start(bo_sb[:], bo_bc[:, :])
        lneps_sb = p_const.tile([1, 1], f32, tag="lneps")
        nc.gpsimd.memset(lneps_sb[:], LNEPS)
        wq_sb = []
        for t in range(ND):
            w1 = p_wq.tile([P, D_], f32r, tag="wq")
            nc.sync.dma_start(w1[:], wqT[t * P:(t + 1) * P, :])
            wq_sb.append(w1)

        # kv accumulator [65, H*R] — scoped so it frees after the DMA out
        cc_in = p_dram.tile([W65, H_ * R_], f32, tag="ccin")
        cc_out = p_dram.tile([2 * W65, H_ * R_], f32, tag="ccout")
        with (
            tc.tile_pool(name="kvloc", bufs=1) as p_kvloc,
            tc.tile_pool(name="wk", bufs=ND) as p_wk,
            tc.tile_pool(name="wv", bufs=ND) as p_wv,
            tc.tile_pool(name="rfa", bufs=ND) as p_rfa,
            tc.tile_pool(name="xk", bufs=ND) as p_x,
            tc.tile_pool(name="ktc", bufs=ND) as p_kt,
            tc.tile_pool(name="vt", bufs=NM) as p_v,
            tc.tile_pool(name="kp", bufs=4) as p_kp,
            tc.tile_pool(name="ksm", bufs=6) as p_ksm,
        ):
            kv_sb = p_kvloc.tile([W65, H_ * R_], f32, tag="kvloc")
            nc.gpsimd.memset(kv_sb[:], 0.0)
            wk_sb = []
            wv_sb = []
            rfa_sb = []
            for t in range(ND):
                w1 = p_wk.tile([P, D_], f32r, tag="wk")
                nc.sync.dma_start(w1[:], wkT[t * P:(t + 1) * P, :])
                wk_sb.append(w1)
                w2 = p_wv.tile([P, D_], f32r, tag="wv")
                nc.sync.dma_start(w2[:], wvT[t * P:(t + 1) * P, :])
                wv_sb.append(w2)
                rr = p_rfa.tile([P, 2 * (R_ + HD_)], f32r, tag="rfa")
                nc.sync.dma_start(rr[:], rfa2[t * P:(t + 1) * P, :])
                rfa_sb.append(rr)

            for ch in range(NC_):
                c0 = ch * CH
                x_sb = []
                for t in range(ND):
                    xt = p_x.tile([P, CH], f32r, tag="xk")
                    nc.sync.dma_start(xt[:], xT[t * P:(t + 1) * P, c0:c0 + CH])
                    x_sb.append(xt)
                # kT projection (feature-major)
                kt_sb = []
                for nt in range(ND):
                    ps = pp.tile([P, CH], f32, tag="ps")
                    for kt in range(ND):
                        mm(ps[:], r(wk_sb[kt][:, nt * P:(nt + 1) * P]),
                           r(x_sb[kt][:]), start=(kt == 0), stop=(kt == ND - 1))
                    ktt = p_kt.tile([P, CH], f32r, tag="ktc")
                    nc.vector.tensor_scalar_add(ktt[:], ps[:], bk_sb[:, nt:nt + 1])
                    kt_sb.append(ktt)
                # v projection (seq-major, scattered into 65-wide head slots)
                v_sb = []
                for mt in range(NM):
                    vt = p_v.tile([P, H_ * W65], f32r, tag="vt")
                    for nch in range(NNCH):
                        ps = pp.tile([P, NW], f32, tag="ps")
                        for kt in range(ND):
                            mm(ps[:], r(x_sb[kt][:, mt * P:(mt + 1) * P]),
                               r(wv_sb[kt][:, nch * NW:(nch + 1) * NW]),
                               start=(kt == 0), stop=(kt == ND - 1))
                        hpc = NW // HD_  # heads per n-chunk (8)
                        ov = vt[:].rearrange("p (h w) -> p h w", w=W65)[
                            :, nch * hpc:(nch + 1) * hpc, 0:HD_]
                        iv = ps[:].rearrange("p (h w) -> p h w", w=HD_)
                        bb = bv_sb[:].rearrange("p (h w) -> p h w", w=W65)[
                            :, nch * hpc:(nch + 1) * hpc, 0:HD_]
                        nc.vector.tensor_tensor(ov, iv, bb, OP.add)
                    # ones columns (from bv_bc, which holds 1.0 at slot col 64)
                    oo = vt[:].rearrange("p (h w) -> p h w", w=W65)[:, :, HD_:W65]
                    bo1 = bv_sb[:].rearrange("p (h w) -> p h w", w=W65)[:, :, HD_:W65]
                    nc.vector.tensor_copy(oo, bo1)
                    v_sb.append(vt)
                # heads: features, exp, kv accumulation
                for j in range(ND):
                    for hh in range(2):
                        h = 2 * j + hh
                        kvp = pp.tile([W65, R_], f32, tag="ps")
                        for mt in range(NM):
                            kfa = pp.tile([P, R_ + HD_], f32, tag="ps")
                            mm(kfa[:], r(kt_sb[j][:, mt * P:(mt + 1) * P]),
                               r(rfa_sb[j][:, hh * (R_ + HD_):(hh + 1) * (R_ + HD_)]),
                               start=True, stop=True)
                            sqs = p_ksm.tile([P, HD_], f32, tag="sqs")
                            ksq = p_ksm.tile([P, 1], f32, tag="ksq")
                            nc.scalar.activation(sqs[:], kfa[:, R_:R_ + HD_],
                                                 AF.Square, scale=SQH,
                                                 accum_out=ksq[:])
                            nksq = p_ksm.tile([P, 1], f32, tag="nksq")
                            nc.gpsimd.tensor_scalar_mul(nksq[:], ksq[:], -1.0)
                            kpt = p_kp.tile([P, R_], f32r, tag="kp")
                            nc.scalar.activation(kpt[:], kfa[:, 0:R_], AF.Exp,
                                                 bias=nksq[:, 0:1])
                            mm(kvp[:], r(v_sb[mt][:, h * W65:(h + 1) * W65]),
                               r(kpt[:]), start=(mt == 0), stop=(mt == NM - 1))
                        nc.vector.tensor_tensor(
                            kv_sb[:, h * R_:(h + 1) * R_],
                            kv_sb[:, h * R_:(h + 1) * R_], kvp[:], OP.add)

            nc.sync.dma_start(cc_in[:], kv_sb[:])

        # =================== PAIR ALLGATHER + TRANSPOSE ===================
        nc.gpsimd.collective_compute(
            "AllGather", OP.bypass, replica_groups=c["pairs"],
            ins=[cc_in[:].opt()], outs=[cc_out[:].opt()])
        kvr = p_kvr.tile([P, H_ * RT * W65], f32r, tag="kvr")
        with tc.tile_pool(name="kvg", bufs=1) as p_kvg:
            kva = p_kvg.tile([W65, H_ * R_], f32, tag="kva")
            nc.sync.dma_start(kva[:], cc_out[0:W65, :])
            kvb = p_kvg.tile([W65, H_ * R_], f32, tag="kvb")
            nc.sync.dma_start(kvb[:], cc_out[W65:2 * W65, :])
            nc.vector.tensor_tensor(kva[:], kva[:], kvb[:], OP.add)
            # transpose to r-major: kvr [128(r), (2h+rt)*65]
            for h in range(H_):
                tp = pp.tile([P, RT * W65], f32, tag="ps")
                for rt in range(RT):
                    nc.tensor.transpose(
                        tp[:, rt * W65:(rt + 1) * W65],
                        kva[:, h * R_ + rt * P: h * R_ + (rt + 1) * P],
                        id_sb[0:W65, 0:W65])
                nc.vector.tensor_copy(
                    kvr[:, (RT * h) * W65:(RT * h + RT) * W65], tp[:])

        # =================== PASS Q ===================
        if True:
            with (
                tc.tile_pool(name="rfq", bufs=ND) as p_rfq,
                tc.tile_pool(name="wo", bufs=ND) as p_wo,
                tc.tile_pool(name="xq", bufs=ND + 2) as p_xq,
                tc.tile_pool(name="qtc", bufs=ND) as p_qt,
                tc.tile_pool(name="q2", bufs=3) as p_q2,
                tc.tile_pool(name="qp", bufs=5) as p_qp,
                tc.tile_pool(name="att", bufs=ND) as p_att,
                tc.tile_pool(name="nrm", bufs=3) as p_nrm,
                tc.tile_pool(name="bcp", bufs=2) as p_bc,
                tc.tile_pool(name="oub", bufs=2) as p_ou,
            ):
                rfq_sb = []
                wo_sb = []
                for t in range(ND):
                    rr = p_rfq.tile([P, 2 * R_], f32r, tag="rfq")
                    nc.sync.dma_start(rr[:], rfq2[t * P:(t + 1) * P, :])
                    rfq_sb.append(rr)
                for t in range(ND):
                    w2 = p_wo.tile([P, D_], f32r, tag="wo")
                    nc.sync.dma_start(w2[:], woT[t * P:(t + 1) * P, :])
                    wo_sb.append(w2)

                for ch in range(NC_):
                    c0 = ch * CH
                    x_sb = []
                    for t in range(ND):
                        xt = p_xq.tile([P, CH], f32r, tag="xq")
                        nc.sync.dma_start(xt[:], xT[t * P:(t + 1) * P, c0:c0 + CH])
                        x_sb.append(xt)
                    qt_sb = []
                    for nt in range(ND):
                        ps = pp.tile([P, CH], f32, tag="ps")
                        for kt in range(ND):
                            mm(ps[:], r(wq_sb[kt][:, nt * P:(nt + 1) * P]),
                               r(x_sb[kt][:]), start=(kt == 0), stop=(kt == ND - 1))
                        qtt = p_qt.tile([P, CH], f32r, tag="qtc")
                        nc.scalar.add(qtt[:], ps[:], bq_sb[:, nt:nt + 1])
                        qt_sb.append(qtt)
                    attn_sb = []
                    for j in range(ND):
                        q2t = p_q2.tile([P, CH], f32r, tag="q2")
                        nc.scalar.activation(q2t[:], qt_sb[j][:], AF.Square)
                        att = p_att.tile([P, CH], f32r, tag="att")
                        for hh in range(2):
                            h = 2 * j + hh
                            # q features + exp (no stabilizer)
                            qp_t = []
                            for rt in range(RT):
                                qf = pp.tile([P, CH], f32, tag="ps")
                                mm(qf[:], r(rfq_sb[j][:, hh * R_ + rt * P:
                                                      hh * R_ + (rt + 1) * P]),
                                   r(qt_sb[j][:]), start=True, stop=True)
                                qp = p_qp.tile([P, CH], f32r, tag="qp")
                                nc.scalar.activation(qp[:], qf[:], AF.Exp)
                                qp_t.append(qp)
                            # qsq -> eqsq = 1e-6 * e^{qsq}; park it at partition 64
                            qsq = pp.tile([1, CH], f32, tag="ps")
                            mm(qsq[:], r(hm_sb[:, hh:hh + 1]), r(q2t[:]),
                               start=True, stop=True)
                            eqsq = p_nrm.tile([1, CH], f32, tag="eqsq")
                            nc.scalar.activation(eqsq[:], qsq[:], AF.Exp,
                                                 bias=lneps_sb[0:1, 0:1])
                            # attention + normalizer in one matmul (M=65)
                            ah = pp.tile([W65, CH], f32, tag="ps")
                            for rt in range(RT):
                                mm(ah[:],
                                   r(kvr[:, (RT * h + rt) * W65:
                                         (RT * h + rt) * W65 + W65]),
                                   r(qp_t[rt][:]), start=(rt == 0),
                                   stop=(rt == RT - 1))
                            # denom+recip at partition 64, then DMA the row
                            # down to partition 0 for the broadcast
                            e64 = p_nrm.tile([W65, CH], f32, tag="e64")
                            nc.sync.dma_start(e64[HD_:W65, :], eqsq[:])
                            nc.vector.tensor_tensor(e64[HD_:W65, :],
                                                    ah[HD_:W65, :],
                                                    e64[HD_:W65, :], OP.add)
                            nc.vector.reciprocal(e64[HD_:W65, :], e64[HD_:W65, :])
                            r0 = p_nrm.tile([1, CH], f32, tag="r0")
                            nc.sync.dma_start(r0[:], e64[HD_:W65, :])
                            bcp = p_bc.tile([HD_, CH], f32, tag="bcp")
                            nc.gpsimd.partition_broadcast(bcp[:], r0[:])
                            mul32 = p_bc.tile([HD_, CH], f32, tag="mul32")
                            nc.vector.tensor_tensor(mul32[:], ah[0:HD_, :],
                                                    bcp[:], OP.mult)
                            if hh == 0:
                                nc.gpsimd.tensor_copy(att[0:HD_, :], mul32[:])
                            else:
                                scr = p_bc.tile([HD_, CH], f32r, tag="scr")
                                nc.gpsimd.tensor_copy(scr[:], mul32[:])
                                # partition move 0:64 -> 64:128 via DMA
                                nc.sync.dma_start(att[HD_:P, :], scr[:])
                        attn_sb.append(att)
                    # out projection (seq-major) + bias + store
                    for mt in range(NM):
                        ot = p_ou.tile([P, D_], f32, tag="oub")
                        for nch in range(NNCH):
                            ps = pp.tile([P, NW], f32, tag="ps")
                            for j in range(ND):
                                mm(ps[:], r(attn_sb[j][:, mt * P:(mt + 1) * P]),
                                   r(wo_sb[j][:, nch * NW:(nch + 1) * NW]),
                                   start=(j == 0), stop=(j == ND - 1))
                            nc.vector.tensor_tensor(
                                ot[:, nch * NW:(nch + 1) * NW], ps[:],
                                bo_sb[:, nch * NW:(nch + 1) * NW], OP.add)
                        nc.sync.dma_start(
                            out_d[c0 + mt * P:c0 + (mt + 1) * P, :], ot[:])


def _declare_io(nc, c):
    from concourse import mybir
    f32 = mybir.dt.float32
    D_, H_, R_, HD_, M_ = c["D"], c["H"], c["R"], c["HD"], c["MLOC"]
    f32r = mybir.dt.float32r
    io = {}
    def inp(name, shape, dt=f32):
        io[name] = nc.dram_tensor(name, shape, dt, kind="ExternalInput").ap()
    inp("xT", [D_, M_], f32r)
    inp("wqT", [D_, D_], f32r)
    inp("wkT", [D_, D_], f32r)
    inp("wvT", [D_, D_], f32r)
    inp("woT", [D_, D_], f32r)
    inp("rfa2", [D_, 2 * (R_ + HD_)], f32r)
    inp("rfq2", [D_, 2 * R_], f32r)
    inp("hm_ab", [128, 2], f32r)
    inp("bq_t", [128, D_ // 128])
    inp("bk_t", [128, D_ // 128])
    inp("bv_bc", [128, H_ * (HD_ + 1)])
    inp("bo_bc", [128, D_])
    io["out"] = nc.dram_tensor("out", [M_, D_], f32, kind="ExternalOutput").ap()
    return io


def build_bass(cfg):
    import concourse.tile as tile
    from concourse import bacc
    nc = bacc.Bacc("TRN2", target_bir_lowering=False, debug=False,
                   enable_asserts=False, num_devices=cfg["ncores"])
    io = _declare_io(nc, cfg)
    with tile.TileContext(nc) as tc:
        _emit(tc, io, cfg)
    nc.compile()
    return nc


def host_inputs(x_slice_T, wq, bq, wk, bk, wv, bv, wo, bo, rf, cfg):
    """Build the per-core input map. x_slice_T: [D, MLOC] for this core."""
    D_, H_, R_, HD_ = cfg["D"], cfg["H"], cfg["R"], cfg["HD"]
    ND = D_ // 128
    f = np.float32
    ey = np.eye(HD_, dtype=f)
    rfa2 = np.zeros((D_, 2 * (R_ + HD_)), f)
    rfq2 = np.zeros((D_, 2 * R_), f)
    for j in range(ND):
        for hh in range(2):
            h = 2 * j + hh
            rows = slice(j * 128 + hh * HD_, j * 128 + (hh + 1) * HD_)
            rfa2[rows, hh * (R_ + HD_):hh * (R_ + HD_) + R_] = rf[h]
            rfa2[rows, hh * (R_ + HD_) + R_:(hh + 1) * (R_ + HD_)] = ey
            rfq2[rows, hh * R_:(hh + 1) * R_] = rf[h]
    hm_ab = np.zeros((128, 2), f)
    hm_ab[0:HD_, 0] = 0.5
    hm_ab[HD_:128, 1] = 0.5
    bv_bc = np.zeros((128, H_ * (HD_ + 1)), f)
    for h in range(H_):
        bv_bc[:, h * (HD_ + 1):h * (HD_ + 1) + HD_] = bv[h * HD_:(h + 1) * HD_][None, :]
        bv_bc[:, h * (HD_ + 1) + HD_] = 1.0
    return {
        "xT": np.ascontiguousarray(x_slice_T, f),
        "wqT": np.ascontiguousarray(wq.T, f),
        "wkT": np.ascontiguousarray(wk.T, f),
        "wvT": np.ascontiguousarray(wv.T, f),
        "woT": np.ascontiguousarray(wo.T, f),
        "rfa2": rfa2, "rfq2": rfq2, "hm_ab": hm_ab,
        "bq_t": np.ascontiguousarray(bq.reshape(ND, 128).T, f),
        "bk_t": np.ascontiguousarray(bk.reshape(ND, 128).T, f),
        "bv_bc": bv_bc,
        "bo_bc": np.ascontiguousarray(np.tile(bo[None, :], (128, 1)), f),
    }


_NC_CACHE = {}
LAST_RESULTS = None


def kernel(**inputs):
    global LAST_RESULTS
    from concourse.bass_utils import run_bass_kernel_spmd

    cfg = FULL_CFG
    x = np.asarray(inputs["x"], np.float32)
    args = [np.asarray(inputs[k], np.float32) for k in
            ["wq", "bq", "wk", "bk", "wv", "bv", "wo", "bo", "random_features"]]

    key = "full"
    if key not in _NC_CACHE:
        _NC_CACHE[key] = build_bass(cfg)
    nc = _NC_CACHE[key]

    base = host_inputs(np.zeros((cfg["D"], cfg["MLOC"]), np.float32), *args, cfg)
    in_maps = []
    for c in range(cfg["ncores"]):
        b, half = c // 2, c % 2
        m = dict(base)
        m["xT"] = np.ascontiguousarray(
            x[b, half * cfg["MLOC"]:(half + 1) * cfg["MLOC"], :].T)
        in_maps.append(m)

    trace = os.environ.get("KBENCH_TRACE", "0") == "1"
    res = run_bass_kernel_spmd(
        nc, in_maps, core_ids=list(range(cfg["ncores"])), trace=trace)
    LAST_RESULTS = res
    out = np.concatenate([res.results[c]["out"] for c in range(cfg["ncores"])],
                         axis=0)
    return out.reshape(B, S, D).astype(np.float32)


# revision 23
# speedup vs baseline: 1.1432x; 1.0223x over previous
"""Performer attention TRN2 Bass kernel.

Strategy: sequence-parallel over the 8 cores (each core owns 2048 rows =
half of one batch; cores 2i,2i+1 share batch i). The Performer kv
aggregation sums over the full sequence, so the two cores of a pair
AllGather their partial kv matrices (tiny: H*65*R fp32 ~ 1MB) and sum.
Everything else is fully local.

Math restructuring vs the reference (exactly equivalent in real
arithmetic): q_prime is computed WITHOUT the -0.5|q|^2 stabilizer; the
factor e^{-qsq} cancels between numerator and normalizer, except in the
+1e-6 term, which is compensated by using denominator
(Nu + 1e-6 * e^{qsq}).  k_prime keeps its stabilizer (it is inside the
sequence sum).

All matmuls run as float32r (fp22 multiplies, fp32 accumulate) which is
full PE speed for moving dim >= 256.

Layouts (host pre-transposed, see kernel()):
  xT   [D, MLOC]   feature-major activations
  w*T  [D, D]      transposed weights
  rfa2 [D, 2*(R+HD)] per d-tile j: block-diag [rf_{2j}|I64] / [rf_{2j+1}|I64]
  rfq2 [D, 2*R]      per d-tile j: block-diag rf_{2j} / rf_{2j+1}
"""

import os
import sys

import numpy as np

for _p in ("/opt/trn_rl_repo", "/opt/pypackages"):
    if _p not in sys.path:
        sys.path.append(_p)

B, S, D, H, R, HD = 4, 4096, 1024, 16, 256, 64
NCORES = 8
MLOC = (B * S) // NCORES  # 2048

FULL_CFG = dict(
    D=D, H=H, R=R, HD=HD, MLOC=MLOC, CHUNK=512,
    ncores=NCORES, pairs=[[0, 1], [2, 3], [4, 5], [6, 7]],
)


def _emit(tc, io, c):
    import concourse.bass as bass  # noqa: F401
    from concourse import mybir

    nc = tc.nc
    f32 = mybir.dt.float32
    f32r = mybir.dt.float32r
    AF = mybir.ActivationFunctionType
    OP = mybir.AluOpType
    P = 128

    D_, H_, R_, HD_ = c["D"], c["H"], c["R"], c["HD"]
    M_, CH = c["MLOC"], c["CHUNK"]
    ND = D_ // P          # d-tiles (= head pairs)
    NM = CH // P          # m-tiles per chunk
    NC_ = M_ // CH        # chunks
    RT = R_ // P          # r-tiles per head
    NW = c.get("NW", min(512, D_))  # n-chunk width
    NNCH = D_ // NW       # n chunks
    W65 = HD_ + 1         # 65
    SQH = float(np.sqrt(0.5))
    LNEPS = float(np.log(1e-6))

    def r(ap):
        return ap.bitcast(f32r)

    mm = nc.tensor.matmul

    xT, wqT, wkT, wvT, woT = io["xT"], io["wqT"], io["wkT"], io["wvT"], io["woT"]
    rfa2, rfq2, hm_ab = io["rfa2"], io["rfq2"], io["hm_ab"]
    bq_t, bk_t, bv_bc, bo_bc = io["bq_t"], io["bk_t"], io["bv_bc"], io["bo_bc"]
    out_d = io["out"]

    from contextlib import ExitStack
    with (
        tc.tile_pool(name="const", bufs=1) as p_const,
        tc.tile_pool(name="kvrp", bufs=1) as p_kvr,
        tc.tile_pool(name="wq", bufs=ND) as p_wq,
        tc.tile_pool(name="psum", bufs=8, space="PSUM") as pp,
        tc.tile_pool(name="dram", bufs=1, space="DRAM") as p_dram,
    ):
        # ---- persistent constants ----
        id_sb = p_const.tile([P, P], f32, tag="ident")
        from concourse.masks import make_identity
        make_identity(nc, id_sb[:])
        hm_sb = p_const.tile([P, 2], f32r, tag="hm")
        nc.sync.dma_start(hm_sb[:], hm_ab[:, :])
        bq_sb = p_const.tile([P, ND], f32, tag="bq")
        nc.sync.dma_start(bq_sb[:], bq_t[:, :])
        bk_sb = p_const.tile([P, ND], f32, tag="bk")
        nc.sync.dma_start(bk_sb[:], bk_t[:, :])
        bv_sb = p_const.tile([P, H_ * W65], f32, tag="bv")
        nc.sync.dma_start(bv_sb[:], bv_bc[:, :])
        bo_sb = p_const.tile([P, D_], f32, tag="bo")
        nc.sync.dma_start(bo_sb[:], bo_bc[:, :])
        lneps_sb = p_const.tile([1, 1], f32, tag="lneps")
        nc.gpsimd.memset(lneps_sb[:], LNEPS)
        wq_sb = []
        for t in range(ND):
            w1 = p_wq.tile([P, D_], f32r, tag="wq")
            nc.sync.dma_start(w1[:], wqT[t * P:(t + 1) * P, :])
            wq_sb.append(w1)

        # kv accumulator [65, H*R] — scoped so it frees after the DMA out
        cc_in = p_dram.tile([W65, H_ * R_], f32, tag="ccin")
        cc_out = p_dram.tile([2 * W65, H_ * R_], f32, tag="ccout")
        with (
            tc.tile_pool(name="kvloc", bufs=1) as p_kvloc,
            tc.tile_pool(name="wk", bufs=ND) as p_wk,
            tc.tile_pool(name="wv", bufs=ND) as p_wv,
            tc.tile_pool(name="rfa", bufs=ND) as p_rfa,
            tc.tile_pool(name="xk", bufs=ND) as p_x,
            tc.tile_pool(name="ktc", bufs=ND) as p_kt,
            tc.tile_pool(name="vt", bufs=NM) as p_v,
            tc.tile_pool(name="kp", bufs=4) as p_kp,
            tc.tile_pool(name="ksm", bufs=6) as p_ksm,
        ):
            kv_sb = p_kvloc.tile([W65, H_ * R_], f32, tag="kvloc")
            nc.gpsimd.memset(kv_sb[:], 0.0)
            wk_sb = []
            wv_sb = []
            rfa_sb = []
            for t in range(ND):
                w1 = p_wk.tile([P, D_], f32r, tag="wk")
                nc.sync.dma_start(w1[:], wkT[t * P:(t + 1) * P, :])
                wk_sb.append(w1)
                w2 = p_wv.tile([P, D_], f32r, tag="wv")
                nc.sync.dma_start(w2[:], wvT[t * P:(t + 1) * P, :])
                wv_sb.append(w2)
                rr = p_rfa.tile([P, 2 * (R_ + HD_)], f32r, tag="rfa")
                nc.sync.dma_start(rr[:], rfa2[t * P:(t + 1) * P, :])
                rfa_sb.append(rr)

            for ch in range(NC_):
                c0 = ch * CH
                x_sb = []
                for t in range(ND):
                    xt = p_x.tile([P, CH], f32r, tag="xk")
                    nc.sync.dma_start(xt[:], xT[t * P:(t + 1) * P, c0:c0 + CH])
                    x_sb.append(xt)
                # kT projection (feature-major)
                kt_sb = []
                for nt in range(ND):
                    ps = pp.tile([P, CH], f32, tag="ps")
                    for kt in range(ND):
                        mm(ps[:], r(wk_sb[kt][:, nt * P:(nt + 1) * P]),
                           r(x_sb[kt][:]), start=(kt == 0), stop=(kt == ND - 1))
                    ktt = p_kt.tile([P, CH], f32r, tag="ktc")
                    nc.vector.tensor_scalar_add(ktt[:], ps[:], bk_sb[:, nt:nt + 1])
                    kt_sb.append(ktt)
                # v projection (seq-major, scattered into 65-wide head slots)
                v_sb = []
                for mt in range(NM):
                    vt = p_v.tile([P, H_ * W65], f32r, tag="vt")
                    for nch in range(NNCH):
                        ps = pp.tile([P, NW], f32, tag="ps")
                        for kt in range(ND):
                            mm(ps[:], r(x_sb[kt][:, mt * P:(mt + 1) * P]),
                               r(wv_sb[kt][:, nch * NW:(nch + 1) * NW]),
                               start=(kt == 0), stop=(kt == ND - 1))
                        hpc = NW // HD_  # heads per n-chunk (8)
                        ov = vt[:].rearrange("p (h w) -> p h w", w=W65)[
                            :, nch * hpc:(nch + 1) * hpc, 0:HD_]
                        iv = ps[:].rearrange("p (h w) -> p h w", w=HD_)
                        bb = bv_sb[:].rearrange("p (h w) -> p h w", w=W65)[
                            :, nch * hpc:(nch + 1) * hpc, 0:HD_]
                        nc.vector.tensor_tensor(ov, iv, bb, OP.add)
                    # ones columns (from bv_bc, which holds 1.0 at slot col 64)
                    oo = vt[:].rearrange("p (h w) -> p h w", w=W65)[:, :, HD_:W65]
                    bo1 = bv_sb[:].rearrange("p (h w) -> p h w", w=W65)[:, :, HD_:W65]
                    nc.vector.tensor_copy(oo, bo1)
                    v_sb.append(vt)
                # heads: features, exp, kv accumulation
                for j in range(ND):
                    for hh in range(2):
                        h = 2 * j + hh
                        kvp = pp.tile([W65, R_], f32, tag="ps")
                        for mt in range(NM):
                            kfa = pp.tile([P, R_ + HD_], f32, tag="ps")
                            mm(kfa[:], r(kt_sb[j][:, mt * P:(mt + 1) * P]),
                               r(rfa_sb[j][:, hh * (R_ + HD_):(hh + 1) * (R_ + HD_)]),
                               start=True, stop=True)
                            sqs = p_ksm.tile([P, HD_], f32, tag="sqs")
                            ksq = p_ksm.tile([P, 1], f32, tag="ksq")
                            nc.scalar.activation(sqs[:], kfa[:, R_:R_ + HD_],
                                                 AF.Square, scale=SQH,
                                                 accum_out=ksq[:])
                            nksq = p_ksm.tile([P, 1], f32, tag="nksq")
                            nc.gpsimd.tensor_scalar_mul(nksq[:], ksq[:], -1.0)
                            kpt = p_kp.tile([P, R_], f32r, tag="kp")
                            nc.scalar.activation(kpt[:], kfa[:, 0:R_], AF.Exp,
                                                 bias=nksq[:, 0:1])
                            mm(kvp[:], r(v_sb[mt][:, h * W65:(h + 1) * W65]),
                               r(kpt[:]), start=(mt == 0), stop=(mt == NM - 1))
                        nc.vector.tensor_tensor(
                            kv_sb[:, h * R_:(h + 1) * R_],
                            kv_sb[:, h * R_:(h + 1) * R_], kvp[:], OP.add)

            nc.sync.dma_start(cc_in[:], kv_sb[:])

        # =================== PAIR ALLGATHER + TRANSPOSE ===================
        nc.gpsimd.collective_compute(
            "AllGather", OP.bypass, replica_groups=c["pairs"],
            ins=[cc_in[:].opt()], outs=[cc_out[:].opt()])
        kvr = p_kvr.tile([P, H_ * RT * W65], f32r, tag="kvr")
        with tc.tile_pool(name="kvg", bufs=1) as p_kvg:
            kva = p_kvg.tile([W65, H_ * R_], f32, tag="kva")
            nc.sync.dma_start(kva[:], cc_out[0:W65, :])
            kvb = p_kvg.tile([W65, H_ * R_], f32, tag="kvb")
            nc.sync.dma_start(kvb[:], cc_out[W65:2 * W65, :])
            nc.vector.tensor_tensor(kva[:], kva[:], kvb[:], OP.add)
            # transpose to r-major: kvr [128(r), (2h+rt)*65]
            for h in range(H_):
                tp = pp.tile([P, RT * W65], f32, tag="ps")
                for rt in range(RT):
                    nc.tensor.transpose(
                        tp[:, rt * W65:(rt + 1) * W65],
                        kva[:, h * R_ + rt * P: h * R_ + (rt + 1) * P],
                        id_sb[0:W65, 0:W65])
                nc.vector.tensor_copy(
                    kvr[:, (RT * h) * W65:(RT * h + RT) * W65], tp[:])

        # =================== PASS Q ===================
        if True:
            with (
                tc.tile_pool(name="rfq", bufs=ND) as p_rfq,
                tc.tile_pool(name="wo", bufs=ND) as p_wo,
                tc.tile_pool(name="xq", bufs=ND + 2) as p_xq,
                tc.tile_pool(name="qtc", bufs=ND) as p_qt,
                tc.tile_pool(name="q2", bufs=3) as p_q2,
                tc.tile_pool(name="qp", bufs=5) as p_qp,
                tc.tile_pool(name="att", bufs=ND) as p_att,
                tc.tile_pool(name="nrm", bufs=3) as p_nrm,
                tc.tile_pool(name="bcp", bufs=2) as p_bc,
                tc.tile_pool(name="oub", bufs=2) as p_ou,
            ):
                rfq_sb = []
                wo_sb = []
                for t in range(ND):
                    rr = p_rfq.tile([P, 2 * R_], f32r, tag="rfq")
                    nc.sync.dma_start(rr[:], rfq2[t * P:(t + 1) * P, :])
                    rfq_sb.append(rr)
                for t in range(ND):
                    w2 = p_wo.tile([P, D_], f32r, tag="wo")
                    nc.sync.dma_start(w2[:], woT[t * P:(t + 1) * P, :])
                    wo_sb.append(w2)

                for ch in range(NC_):
                    c0 = ch * CH
                    x_sb = []
                    for t in range(ND):
                        xt = p_xq.tile([P, CH], f32r, tag="xq")
                        nc.sync.dma_start(xt[:], xT[t * P:(t + 1) * P, c0:c0 + CH])
                        x_sb.append(xt)
                    qt_sb = []
                    for nt in range(ND):
                        ps = pp.tile([P, CH], f32, tag="ps")
                        for kt in range(ND):
                            mm(ps[:], r(wq_sb[kt][:, nt * P:(nt + 1) * P]),
                               r(x_sb[kt][:]), start=(kt == 0), stop=(kt == ND - 1))
                        qtt = p_qt.tile([P, CH], f32r, tag="qtc")
                        nc.scalar.add(qtt[:], ps[:], bq_sb[:, nt:nt + 1])
                        qt_sb.append(qtt)
                    attn_sb = []
                    for j in range(ND):
                        q2t = p_q2.tile([P, CH], f32r, tag="q2")
                        nc.scalar.activation(q2t[:], qt_sb[j][:], AF.Square)
                        att = p_att.tile([P, CH], f32r, tag="att")
                        for hh in range(2):
                            h = 2 * j + hh
                            # q features + exp (no stabilizer)
                            qp_t = []
                            for rt in range(RT):
                                qf = pp.tile([P, CH], f32, tag="ps")
                                mm(qf[:], r(rfq_sb[j][:, hh * R_ + rt * P:
                                                      hh * R_ + (rt + 1) * P]),
                                   r(qt_sb[j][:]), start=True, stop=True)
                                qp = p_qp.tile([P, CH], f32r, tag="qp")
                                nc.scalar.activation(qp[:], qf[:], AF.Exp)
                                qp_t.append(qp)
                            # qsq -> eqsq = 1e-6 * e^{qsq}; park it at partition 64
                            qsq = pp.tile([1, CH], f32, tag="ps")
                            mm(qsq[:], r(hm_sb[:, hh:hh + 1]), r(q2t[:]),
                               start=True, stop=True)
                            eqsq = p_nrm.tile([1, CH], f32, tag="eqsq")
                            nc.scalar.activation(eqsq[:], qsq[:], AF.Exp,
                                                 bias=lneps_sb[0:1, 0:1])
                            # attention + normalizer in one matmul (M=65)
                            ah = pp.tile([W65, CH], f32, tag="ps")
                            for rt in range(RT):
                                mm(ah[:],
                                   r(kvr[:, (RT * h + rt) * W65:
                                         (RT * h + rt) * W65 + W65]),
                                   r(qp_t[rt][:]), start=(rt == 0),
                                   stop=(rt == RT - 1))
                            # denom+recip at partition 64, then DMA the row
                            # down to partition 0 for the broadcast
                            e64 = p_nrm.tile([W65, CH], f32, tag="e64")
                            nc.sync.dma_start(e64[HD_:W65, :], eqsq[:])
                            nc.vector.tensor_tensor(e64[HD_:W65, :],
                                                    ah[HD_:W65, :],
                                                    e64[HD_:W65, :], OP.add)
                            nc.vector.reciprocal(e64[HD_:W65, :], e64[HD_:W65, :])
                            r0 = p_nrm.tile([1, CH], f32, tag="r0")
                            nc.sync.dma_start(r0[:], e64[HD_:W65, :])
                            bcp = p_bc.tile([HD_, CH], f32, tag="bcp")
                            nc.gpsimd.partition_broadcast(bcp[:], r0[:])
                            if hh == 0:
                                nc.vector.tensor_tensor(att[0:HD_, :],
                                                        ah[0:HD_, :], bcp[:],
                                                        OP.mult)
                            else:
                                scr = p_bc.tile([HD_, CH], f32r, tag="scr")
                                nc.vector.tensor_tensor(scr[:], ah[0:HD_, :],
                                                        bcp[:], OP.mult)
                                # partition move 0:64 -> 64:128 via DMA
                                nc.sync.dma_start(att[HD_:P, :], scr[:])
                        attn_sb.append(att)
                    # out projection (seq-major) + bias + store
                    for mt in range(NM):
                        ot = p_ou.tile([P, D_], f32, tag="oub")
                        for nch in range(NNCH):
                            ps = pp.tile([P, NW], f32, tag="ps")
                            for j in range(ND):
                                mm(ps[:], r(attn_sb[j][:, mt * P:(mt + 1) * P]),
                                   r(wo_sb[j][:, nch * NW:(nch + 1) * NW]),
                                   start=(j == 0), stop=(j == ND - 1))
                            nc.vector.tensor_tensor(
                                ot[:, nch * NW:(nch + 1) * NW], ps[:],
                                bo_sb[:, nch * NW:(nch + 1) * NW], OP.add)
                        nc.sync.dma_start(
                            out_d[c0 + mt * P:c0 + (mt + 1) * P, :], ot[:])


def _declare_io(nc, c):
    from concourse import mybir
    f32 = mybir.dt.float32
    D_, H_, R_, HD_, M_ = c["D"], c["H"], c["R"], c["HD"], c["MLOC"]
    f32r = mybir.dt.float32r
    io = {}
    def inp(name, shape, dt=f32):
        io[name] = nc.dram_tensor(name, shape, dt, kind="ExternalInput").ap()
    inp("xT", [D_, M_], f32r)
    inp("wqT", [D_, D_], f32r)
    inp("wkT", [D_, D_], f32r)
    inp("wvT", [D_, D_], f32r)
    inp("woT", [D_, D_], f32r)
    inp("rfa2", [D_, 2 * (R_ + HD_)], f32r)
    inp("rfq2", [D_, 2 * R_], f32r)
    inp("hm_ab", [128, 2], f32r)
    inp("bq_t", [128, D_ // 128])
    inp("bk_t", [128, D_ // 128])
    inp("bv_bc", [128, H_ * (HD_ + 1)])
    inp("bo_bc", [128, D_])
    io["out"] = nc.dram_tensor("out", [M_, D_], f32, kind="ExternalOutput").ap()
    return io


def build_bass(cfg):
    import concourse.tile as tile
    from concourse import bacc
    nc = bacc.Bacc("TRN2", target_bir_lowering=False, debug=False,
                   enable_asserts=False, num_devices=cfg["ncores"])
    io = _declare_io(nc, cfg)
    with tile.TileContext(nc) as tc:
        _emit(tc, io, cfg)
    nc.compile()
    return nc


def host_inputs(x_slice_T, wq, bq, wk, bk, wv, bv, wo, bo, rf, cfg):
    """Build the per-core input map. x_slice_T: [D, MLOC] for this core."""
    D_, H_, R_, HD_ = cfg["D"], cfg["H"], cfg["R"], cfg["HD"]
    ND = D_ // 128
    f = np.float32
    ey = np.eye(HD_, dtype=f)
    rfa2 = np.zeros((D_, 2 * (R_ + HD_)), f)
    rfq2 = np.zeros((D_, 2 * R_), f)
    for j in range(ND):
        for hh in range(2):
            h = 2 * j + hh
            rows = slice(j * 128 + hh * HD_, j * 128 + (hh + 1) * HD_)
            rfa2[rows, hh * (R_ + HD_):hh * (R_ + HD_) + R_] = rf[h]
            rfa2[rows, hh * (R_ + HD_) + R_:(hh + 1) * (R_ + HD_)] = ey
            rfq2[rows, hh * R_:(hh + 1) * R_] = rf[h]
    hm_ab = np.zeros((128, 2), f)
    hm_ab[0:HD_, 0] = 0.5
    hm_ab[HD_:128, 1] = 0.5
    bv_bc = np.zeros((128, H_ * (HD_ + 1)), f)
    for h in range(H_):
        bv_bc[:, h * (HD_ + 1):h * (HD_ + 1) + HD_] = bv[h * HD_:(h + 1) * HD_][None, :]
        bv_bc[:, h * (HD_ + 1) + HD_] = 1.0
    return {
        "xT": np.ascontiguousarray(x_slice_T, f),
        "wqT": np.ascontiguousarray(wq.T, f),
        "wkT": np.ascontiguousarray(wk.T, f),
        "wvT": np.ascontiguousarray(wv.T, f),
        "woT": np.ascontiguousarray(wo.T, f),
        "rfa2": rfa2, "rfq2": rfq2, "hm_ab": hm_ab,
        "bq_t": np.ascontiguousarray(bq.reshape(ND, 128).T, f),
        "bk_t": np.ascontiguousarray(bk.reshape(ND, 128).T, f),
        "bv_bc": bv_bc,
        "bo_bc": np.ascontiguousarray(np.tile(bo[None, :], (128, 1)), f),
    }


_NC_CACHE = {}
LAST_RESULTS = None


def kernel(**inputs):
    global LAST_RESULTS
    from concourse.bass_utils import run_bass_kernel_spmd

    cfg = FULL_CFG
    x = np.asarray(inputs["x"], np.float32)
    args = [np.asarray(inputs[k], np.float32) for k in
            ["wq", "bq", "wk", "bk", "wv", "bv", "wo", "bo", "random_features"]]

    key = "full"
    if key not in _NC_CACHE:
        _NC_CACHE[key] = build_bass(cfg)
    nc = _NC_CACHE[key]

    base = host_inputs(np.zeros((cfg["D"], cfg["MLOC"]), np.float32), *args, cfg)
    in_maps = []
    for c in range(cfg["ncores"]):
        b, half = c // 2, c % 2
        m = dict(base)
        m["xT"] = np.ascontiguousarray(
            x[b, half * cfg["MLOC"]:(half + 1) * cfg["MLOC"], :].T)
        in_maps.append(m)

    trace = os.environ.get("KBENCH_TRACE", "0") == "1"
    res = run_bass_kernel_spmd(
        nc, in_maps, core_ids=list(range(cfg["ncores"])), trace=trace)
    LAST_RESULTS = res
    out = np.concatenate([res.results[c]["out"] for c in range(cfg["ncores"])],
                         axis=0)
    return out.reshape(B, S, D).astype(np.float32)


# revision 24
# speedup vs baseline: 1.1568x; 1.0119x over previous
"""Performer attention TRN2 Bass kernel.

Strategy: sequence-parallel over the 8 cores (each core owns 2048 rows =
half of one batch; cores 2i,2i+1 share batch i). The Performer kv
aggregation sums over the full sequence, so the two cores of a pair
AllGather their partial kv matrices (tiny: H*65*R fp32 ~ 1MB) and sum.
Everything else is fully local.

Math restructuring vs the reference (exactly equivalent in real
arithmetic): q_prime is computed WITHOUT the -0.5|q|^2 stabilizer; the
factor e^{-qsq} cancels between numerator and normalizer, except in the
+1e-6 term, which is compensated by using denominator
(Nu + 1e-6 * e^{qsq}).  k_prime keeps its stabilizer (it is inside the
sequence sum).

All matmuls run as float32r (fp22 multiplies, fp32 accumulate) which is
full PE speed for moving dim >= 256.

Layouts (host pre-transposed, see kernel()):
  xT   [D, MLOC]   feature-major activations
  w*T  [D, D]      transposed weights
  rfa2 [D, 2*(R+HD)] per d-tile j: block-diag [rf_{2j}|I64] / [rf_{2j+1}|I64]
  rfq2 [D, 2*R]      per d-tile j: block-diag rf_{2j} / rf_{2j+1}
"""

import os
import sys

import numpy as np

for _p in ("/opt/trn_rl_repo", "/opt/pypackages"):
    if _p not in sys.path:
        sys.path.append(_p)

B, S, D, H, R, HD = 4, 4096, 1024, 16, 256, 64
NCORES = 8
MLOC = (B * S) // NCORES  # 2048

FULL_CFG = dict(
    D=D, H=H, R=R, HD=HD, MLOC=MLOC, CHUNK=512,
    ncores=NCORES, pairs=[[0, 1], [2, 3], [4, 5], [6, 7]],
)


def _emit(tc, io, c):
    import concourse.bass as bass  # noqa: F401
    from concourse import mybir

    nc = tc.nc
    f32 = mybir.dt.float32
    f32r = mybir.dt.float32r
    AF = mybir.ActivationFunctionType
    OP = mybir.AluOpType
    P = 128

    D_, H_, R_, HD_ = c["D"], c["H"], c["R"], c["HD"]
    M_, CH = c["MLOC"], c["CHUNK"]
    ND = D_ // P          # d-tiles (= head pairs)
    NM = CH // P          # m-tiles per chunk
    NC_ = M_ // CH        # chunks
    RT = R_ // P          # r-tiles per head
    NW = c.get("NW", min(512, D_))  # n-chunk width
    NNCH = D_ // NW       # n chunks
    W65 = HD_ + 1         # 65
    SQH = float(np.sqrt(0.5))
    LNEPS = float(np.log(1e-6))

    def r(ap):
        return ap.bitcast(f32r)

    mm = nc.tensor.matmul

    xT, wqT, wkT, wvT, woT = io["xT"], io["wqT"], io["wkT"], io["wvT"], io["woT"]
    rfa2, rfq2, hm_ab = io["rfa2"], io["rfq2"], io["hm_ab"]
    bq_t, bk_t, bv_bc, bo_bc = io["bq_t"], io["bk_t"], io["bv_bc"], io["bo_bc"]
    out_d = io["out"]

    from contextlib import ExitStack
    with (
        tc.tile_pool(name="const", bufs=1) as p_const,
        tc.tile_pool(name="kvrp", bufs=1) as p_kvr,
        tc.tile_pool(name="wq", bufs=ND) as p_wq,
        tc.tile_pool(name="psum", bufs=8, space="PSUM") as pp,
        tc.tile_pool(name="dram", bufs=1, space="DRAM") as p_dram,
    ):
        # ---- persistent constants ----
        id_sb = p_const.tile([P, P], f32, tag="ident")
        from concourse.masks import make_identity
        make_identity(nc, id_sb[:])
        hm_sb = p_const.tile([P, 2], f32r, tag="hm")
        nc.sync.dma_start(hm_sb[:], hm_ab[:, :])
        bq_sb = p_const.tile([P, ND], f32, tag="bq")
        nc.sync.dma_start(bq_sb[:], bq_t[:, :])
        bk_sb = p_const.tile([P, ND], f32, tag="bk")
        nc.sync.dma_start(bk_sb[:], bk_t[:, :])
        bv_sb = p_const.tile([P, H_ * W65], f32, tag="bv")
        nc.sync.dma_start(bv_sb[:], bv_bc[:, :])
        bo_sb = p_const.tile([P, D_], f32, tag="bo")
        nc.sync.dma_start(bo_sb[:], bo_bc[:, :])
        lneps_sb = p_const.tile([1, 1], f32, tag="lneps")
        nc.gpsimd.memset(lneps_sb[:], LNEPS)
        wq_sb = []
        for t in range(ND):
            w1 = p_wq.tile([P, D_], f32r, tag="wq")
            nc.sync.dma_start(w1[:], wqT[t * P:(t + 1) * P, :])
            wq_sb.append(w1)

        # kv accumulator [65, H*R] — scoped so it frees after the DMA out
        cc_in = p_dram.tile([W65, H_ * R_], f32, tag="ccin")
        cc_out = p_dram.tile([2 * W65, H_ * R_], f32, tag="ccout")
        with (
            tc.tile_pool(name="kvloc", bufs=1) as p_kvloc,
            tc.tile_pool(name="wk", bufs=ND) as p_wk,
            tc.tile_pool(name="wv", bufs=ND) as p_wv,
            tc.tile_pool(name="rfa", bufs=ND) as p_rfa,
            tc.tile_pool(name="xk", bufs=ND) as p_x,
            tc.tile_pool(name="ktc", bufs=ND) as p_kt,
            tc.tile_pool(name="vt", bufs=NM) as p_v,
            tc.tile_pool(name="kp", bufs=4) as p_kp,
            tc.tile_pool(name="ksm", bufs=6) as p_ksm,
        ):
            kv_sb = p_kvloc.tile([W65, H_ * R_], f32, tag="kvloc")
            nc.gpsimd.memset(kv_sb[:], 0.0)
            wk_sb = []
            wv_sb = []
            rfa_sb = []
            for t in range(ND):
                w1 = p_wk.tile([P, D_], f32r, tag="wk")
                nc.sync.dma_start(w1[:], wkT[t * P:(t + 1) * P, :])
                wk_sb.append(w1)
                w2 = p_wv.tile([P, D_], f32r, tag="wv")
                nc.sync.dma_start(w2[:], wvT[t * P:(t + 1) * P, :])
                wv_sb.append(w2)
                rr = p_rfa.tile([P, 2 * (R_ + HD_)], f32r, tag="rfa")
                nc.sync.dma_start(rr[:], rfa2[t * P:(t + 1) * P, :])
                rfa_sb.append(rr)

            for ch in range(NC_):
                c0 = ch * CH
                x_sb = []
                for t in range(ND):
                    xt = p_x.tile([P, CH], f32r, tag="xk")
                    nc.sync.dma_start(xt[:], xT[t * P:(t + 1) * P, c0:c0 + CH])
                    x_sb.append(xt)
                # kT projection (feature-major)
                kt_sb = []
                for nt in range(ND):
                    ps = pp.tile([P, CH], f32, tag="ps")
                    for kt in range(ND):
                        mm(ps[:], r(wk_sb[kt][:, nt * P:(nt + 1) * P]),
                           r(x_sb[kt][:]), start=(kt == 0), stop=(kt == ND - 1))
                    ktt = p_kt.tile([P, CH], f32r, tag="ktc")
                    nc.vector.tensor_scalar_add(ktt[:], ps[:], bk_sb[:, nt:nt + 1])
                    kt_sb.append(ktt)
                # v projection (seq-major, scattered into 65-wide head slots)
                v_sb = []
                for mt in range(NM):
                    vt = p_v.tile([P, H_ * W65], f32r, tag="vt")
                    for nch in range(NNCH):
                        ps = pp.tile([P, NW], f32, tag="ps")
                        for kt in range(ND):
                            mm(ps[:], r(x_sb[kt][:, mt * P:(mt + 1) * P]),
                               r(wv_sb[kt][:, nch * NW:(nch + 1) * NW]),
                               start=(kt == 0), stop=(kt == ND - 1))
                        hpc = NW // HD_  # heads per n-chunk (8)
                        ov = vt[:].rearrange("p (h w) -> p h w", w=W65)[
                            :, nch * hpc:(nch + 1) * hpc, 0:HD_]
                        iv = ps[:].rearrange("p (h w) -> p h w", w=HD_)
                        bb = bv_sb[:].rearrange("p (h w) -> p h w", w=W65)[
                            :, nch * hpc:(nch + 1) * hpc, 0:HD_]
                        nc.vector.tensor_tensor(ov, iv, bb, OP.add)
                    # ones columns (from bv_bc, which holds 1.0 at slot col 64)
                    oo = vt[:].rearrange("p (h w) -> p h w", w=W65)[:, :, HD_:W65]
                    bo1 = bv_sb[:].rearrange("p (h w) -> p h w", w=W65)[:, :, HD_:W65]
                    nc.vector.tensor_copy(oo, bo1)
                    v_sb.append(vt)
                # heads: features, exp, kv accumulation
                for j in range(ND):
                    for hh in range(2):
                        h = 2 * j + hh
                        kvp = pp.tile([W65, R_], f32, tag="ps")
                        for mt in range(NM):
                            kfa = pp.tile([P, R_ + HD_], f32, tag="ps")
                            mm(kfa[:], r(kt_sb[j][:, mt * P:(mt + 1) * P]),
                               r(rfa_sb[j][:, hh * (R_ + HD_):(hh + 1) * (R_ + HD_)]),
                               start=True, stop=True)
                            sqs = p_ksm.tile([P, HD_], f32, tag="sqs")
                            ksq = p_ksm.tile([P, 1], f32, tag="ksq")
                            nc.scalar.activation(sqs[:], kfa[:, R_:R_ + HD_],
                                                 AF.Square, scale=SQH,
                                                 accum_out=ksq[:])
                            nksq = p_ksm.tile([P, 1], f32, tag="nksq")
                            nc.gpsimd.tensor_scalar_mul(nksq[:], ksq[:], -1.0)
                            kpt = p_kp.tile([P, R_], f32r, tag="kp")
                            nc.scalar.activation(kpt[:], kfa[:, 0:R_], AF.Exp,
                                                 bias=nksq[:, 0:1])
                            mm(kvp[:], r(v_sb[mt][:, h * W65:(h + 1) * W65]),
                               r(kpt[:]), start=(mt == 0), stop=(mt == NM - 1))
                        nc.vector.tensor_tensor(
                            kv_sb[:, h * R_:(h + 1) * R_],
                            kv_sb[:, h * R_:(h + 1) * R_], kvp[:], OP.add)

            nc.sync.dma_start(cc_in[:], kv_sb[:])

        # =================== PAIR ALLGATHER + TRANSPOSE ===================
        nc.gpsimd.collective_compute(
            "AllGather", OP.bypass, replica_groups=c["pairs"],
            ins=[cc_in[:].opt()], outs=[cc_out[:].opt()])
        kvr = p_kvr.tile([P, H_ * RT * W65], f32r, tag="kvr")
        with tc.tile_pool(name="kvg", bufs=1) as p_kvg:
            kva = p_kvg.tile([W65, H_ * R_], f32, tag="kva")
            nc.sync.dma_start(kva[:], cc_out[0:W65, :])
            kvb = p_kvg.tile([W65, H_ * R_], f32, tag="kvb")
            nc.sync.dma_start(kvb[:], cc_out[W65:2 * W65, :])
            nc.vector.tensor_tensor(kva[:], kva[:], kvb[:], OP.add)
            # transpose to r-major: kvr [128(r), (2h+rt)*65]
            for h in range(H_):
                tp = pp.tile([P, RT * W65], f32, tag="ps")
                for rt in range(RT):
                    nc.tensor.transpose(
                        tp[:, rt * W65:(rt + 1) * W65],
                        kva[:, h * R_ + rt * P: h * R_ + (rt + 1) * P],
                        id_sb[0:W65, 0:W65])
                nc.vector.tensor_copy(
                    kvr[:, (RT * h) * W65:(RT * h + RT) * W65], tp[:])

        # =================== PASS Q ===================
        if True:
            with (
                tc.tile_pool(name="rfq", bufs=ND) as p_rfq,
                tc.tile_pool(name="wo", bufs=ND) as p_wo,
                tc.tile_pool(name="xq", bufs=ND + 2) as p_xq,
                tc.tile_pool(name="qtc", bufs=ND) as p_qt,
                tc.tile_pool(name="q2", bufs=3) as p_q2,
                tc.tile_pool(name="qp", bufs=5) as p_qp,
                tc.tile_pool(name="att", bufs=ND) as p_att,
                tc.tile_pool(name="nrm", bufs=3) as p_nrm,
                tc.tile_pool(name="bcp", bufs=2) as p_bc,
                tc.tile_pool(name="oub", bufs=2) as p_ou,
            ):
                rfq_sb = []
                wo_sb = []
                for t in range(ND):
                    rr = p_rfq.tile([P, 2 * R_], f32r, tag="rfq")
                    nc.sync.dma_start(rr[:], rfq2[t * P:(t + 1) * P, :])
                    rfq_sb.append(rr)
                for t in range(ND):
                    w2 = p_wo.tile([P, D_], f32r, tag="wo")
                    nc.sync.dma_start(w2[:], woT[t * P:(t + 1) * P, :])
                    wo_sb.append(w2)

                for ch in range(NC_):
                    c0 = ch * CH
                    x_sb = []
                    for t in range(ND):
                        xt = p_xq.tile([P, CH], f32r, tag="xq")
                        nc.sync.dma_start(xt[:], xT[t * P:(t + 1) * P, c0:c0 + CH])
                        x_sb.append(xt)
                    qt_sb = []
                    for nt in range(ND):
                        ps = pp.tile([P, CH], f32, tag="ps")
                        for kt in range(ND):
                            mm(ps[:], r(wq_sb[kt][:, nt * P:(nt + 1) * P]),
                               r(x_sb[kt][:]), start=(kt == 0), stop=(kt == ND - 1))
                        qtt = p_qt.tile([P, CH], f32r, tag="qtc")
                        nc.scalar.add(qtt[:], ps[:], bq_sb[:, nt:nt + 1])
                        qt_sb.append(qtt)
                    attn_sb = []
                    for j in range(ND):
                        q2t = p_q2.tile([P, CH], f32r, tag="q2")
                        nc.scalar.activation(q2t[:], qt_sb[j][:], AF.Square)
                        att = p_att.tile([P, CH], f32r, tag="att")
                        for hh in range(2):
                            h = 2 * j + hh
                            # q features + exp (no stabilizer)
                            qp_t = []
                            for rt in range(RT):
                                qf = pp.tile([P, CH], f32, tag="ps")
                                mm(qf[:], r(rfq_sb[j][:, hh * R_ + rt * P:
                                                      hh * R_ + (rt + 1) * P]),
                                   r(qt_sb[j][:]), start=True, stop=True)
                                qp = p_qp.tile([P, CH], f32r, tag="qp")
                                nc.scalar.activation(qp[:], qf[:], AF.Exp)
                                qp_t.append(qp)
                            # qsq -> eqsq = 1e-6 * e^{qsq}; park it at partition 64
                            qsq = pp.tile([1, CH], f32, tag="ps")
                            mm(qsq[:], r(hm_sb[:, hh:hh + 1]), r(q2t[:]),
                               start=True, stop=True)
                            eqsq = p_nrm.tile([1, CH], f32, tag="eqsq")
                            nc.scalar.activation(eqsq[:], qsq[:], AF.Exp,
                                                 bias=lneps_sb[0:1, 0:1])
                            # attention + normalizer in one matmul (M=65)
                            ah = pp.tile([W65, CH], f32, tag="ps")
                            for rt in range(RT):
                                mm(ah[:],
                                   r(kvr[:, (RT * h + rt) * W65:
                                         (RT * h + rt) * W65 + W65]),
                                   r(qp_t[rt][:]), start=(rt == 0),
                                   stop=(rt == RT - 1))
                            # denom+recip at partition 64, then DMA the row
                            # down to partition 0 for the broadcast
                            e64 = p_nrm.tile([W65, CH], f32, tag="e64")
                            nc.sync.dma_start(e64[HD_:W65, :], eqsq[:])
                            nc.vector.tensor_tensor(e64[HD_:W65, :],
                                                    ah[HD_:W65, :],
                                                    e64[HD_:W65, :], OP.add)
                            nc.vector.reciprocal(e64[HD_:W65, :], e64[HD_:W65, :])
                            r0 = p_nrm.tile([1, CH], f32, tag="r0")
                            nc.sync.dma_start(r0[:], e64[HD_:W65, :])
                            bcp = p_bc.tile([HD_, CH], f32, tag="bcp")
                            nc.gpsimd.partition_broadcast(bcp[:], r0[:])
                            # DVE writes f32 (f32r DVE writes are ~6x slow);
                            # a gpsimd DMA does the f32->f32r cast and, for the
                            # odd head, the partition move, in one transfer.
                            mul32 = p_bc.tile([HD_, CH], f32, tag="mul32")
                            nc.vector.tensor_tensor(mul32[:], ah[0:HD_, :],
                                                    bcp[:], OP.mult)
                            nc.gpsimd.dma_start(
                                att[hh * HD_:(hh + 1) * HD_, :], mul32[:])
                        attn_sb.append(att)
                    # out projection (seq-major) + bias + store
                    for mt in range(NM):
                        ot = p_ou.tile([P, D_], f32, tag="oub")
                        for nch in range(NNCH):
                            ps = pp.tile([P, NW], f32, tag="ps")
                            for j in range(ND):
                                mm(ps[:], r(attn_sb[j][:, mt * P:(mt + 1) * P]),
                                   r(wo_sb[j][:, nch * NW:(nch + 1) * NW]),
                                   start=(j == 0), stop=(j == ND - 1))
                            nc.vector.tensor_tensor(
                                ot[:, nch * NW:(nch + 1) * NW], ps[:],
                                bo_sb[:, nch * NW:(nch + 1) * NW], OP.add)
                        nc.sync.dma_start(
                            out_d[c0 + mt * P:c0 + (mt + 1) * P, :], ot[:])


def _declare_io(nc, c):
    from concourse import mybir
    f32 = mybir.dt.float32
    D_, H_, R_, HD_, M_ = c["D"], c["H"], c["R"], c["HD"], c["MLOC"]
    f32r = mybir.dt.float32r
    io = {}
    def inp(name, shape, dt=f32):
        io[name] = nc.dram_tensor(name, shape, dt, kind="ExternalInput").ap()
    inp("xT", [D_, M_], f32r)
    inp("wqT", [D_, D_], f32r)
    inp("wkT", [D_, D_], f32r)
    inp("wvT", [D_, D_], f32r)
    inp("woT", [D_, D_], f32r)
    inp("rfa2", [D_, 2 * (R_ + HD_)], f32r)
    inp("rfq2", [D_, 2 * R_], f32r)
    inp("hm_ab", [128, 2], f32r)
    inp("bq_t", [128, D_ // 128])
    inp("bk_t", [128, D_ // 128])
    inp("bv_bc", [128, H_ * (HD_ + 1)])
    inp("bo_bc", [128, D_])
    io["out"] = nc.dram_tensor("out", [M_, D_], f32, kind="ExternalOutput").ap()
    return io


def build_bass(cfg):
    import concourse.tile as tile
    from concourse import bacc
    nc = bacc.Bacc("TRN2", target_bir_lowering=False, debug=False,
                   enable_asserts=False, num_devices=cfg["ncores"])
    io = _declare_io(nc, cfg)
    with tile.TileContext(nc) as tc:
        _emit(tc, io, cfg)
    nc.compile()
    return nc


def host_inputs(x_slice_T, wq, bq, wk, bk, wv, bv, wo, bo, rf, cfg):
    """Build the per-core input map. x_slice_T: [D, MLOC] for this core."""
    D_, H_, R_, HD_ = cfg["D"], cfg["H"], cfg["R"], cfg["HD"]
    ND = D_ // 128
    f = np.float32
    ey = np.eye(HD_, dtype=f)
    rfa2 = np.zeros((D_, 2 * (R_ + HD_)), f)
    rfq2 = np.zeros((D_, 2 * R_), f)
    for j in range(ND):
        for hh in range(2):
            h = 2 * j + hh
            rows = slice(j * 128 + hh * HD_, j * 128 + (hh + 1) * HD_)
            rfa2[rows, hh * (R_ + HD_):hh * (R_ + HD_) + R_] = rf[h]
            rfa2[rows, hh * (R_ + HD_) + R_:(hh + 1) * (R_ + HD_)] = ey
            rfq2[rows, hh * R_:(hh + 1) * R_] = rf[h]
    hm_ab = np.zeros((128, 2), f)
    hm_ab[0:HD_, 0] = 0.5
    hm_ab[HD_:128, 1] = 0.5
    bv_bc = np.zeros((128, H_ * (HD_ + 1)), f)
    for h in range(H_):
        bv_bc[:, h * (HD_ + 1):h * (HD_ + 1) + HD_] = bv[h * HD_:(h + 1) * HD_][None, :]
        bv_bc[:, h * (HD_ + 1) + HD_] = 1.0
    return {
        "xT": np.ascontiguousarray(x_slice_T, f),
        "wqT": np.ascontiguousarray(wq.T, f),
        "wkT": np.ascontiguousarray(wk.T, f),
        "wvT": np.ascontiguousarray(wv.T, f),
        "woT": np.ascontiguousarray(wo.T, f),
        "rfa2": rfa2, "rfq2": rfq2, "hm_ab": hm_ab,
        "bq_t": np.ascontiguousarray(bq.reshape(ND, 128).T, f),
        "bk_t": np.ascontiguousarray(bk.reshape(ND, 128).T, f),
        "bv_bc": bv_bc,
        "bo_bc": np.ascontiguousarray(np.tile(bo[None, :], (128, 1)), f),
    }


_NC_CACHE = {}
LAST_RESULTS = None


def kernel(**inputs):
    global LAST_RESULTS
    from concourse.bass_utils import run_bass_kernel_spmd

    cfg = FULL_CFG
    x = np.asarray(inputs["x"], np.float32)
    args = [np.asarray(inputs[k], np.float32) for k in
            ["wq", "bq", "wk", "bk", "wv", "bv", "wo", "bo", "random_features"]]

    key = "full"
    if key not in _NC_CACHE:
        _NC_CACHE[key] = build_bass(cfg)
    nc = _NC_CACHE[key]

    base = host_inputs(np.zeros((cfg["D"], cfg["MLOC"]), np.float32), *args, cfg)
    in_maps = []
    for c in range(cfg["ncores"]):
        b, half = c // 2, c % 2
        m = dict(base)
        m["xT"] = np.ascontiguousarray(
            x[b, half * cfg["MLOC"]:(half + 1) * cfg["MLOC"], :].T)
        in_maps.append(m)

    trace = os.environ.get("KBENCH_TRACE", "0") == "1"
    res = run_bass_kernel_spmd(
        nc, in_maps, core_ids=list(range(cfg["ncores"])), trace=trace)
    LAST_RESULTS = res
    out = np.concatenate([res.results[c]["out"] for c in range(cfg["ncores"])],
                         axis=0)
    return out.reshape(B, S, D).astype(np.float32)
